# revision 12
# baseline (speedup 1.0000x reference)
"""Trainium2 Bass kernel for nn_Encoder_82910048682485 (binary-tree GNN encoder).

Structure exploited: in the heap-layout complete binary tree, the children of
the contiguous parent range [2^l-1, 2^(l+1)-1) are exactly the contiguous
range [2^(l+1)-1, 2^(l+2)-1), and parent p's children are cols 2s / 2s+1 of
that block.  So the whole computation is a chain of matmuls over shrinking
contiguous blocks — no real gather/scatter.

Sharding: data-parallel over the 8 subtrees rooted at nodes 7..14 (level 3).
Each core owns 2^15 leaves and computes its subtree's 2^16-1 node embeddings.
The top 7 nodes (levels 0..2) are computed on host (7 rows of a 256->128 MLP,
~0.001% of FLOPs).

On-chip layout is transposed: embeddings are stored [EMB=128 partitions,
nodes as free dim].  Then the even/odd child split needed by the cell MLP is
just a stride-2 free-dim access pattern, and each level-up step is 6 PE
matmuls (fp32r) + 3 Lrelu activations.  Leaf chunks stream in and a
binary-counter cascade of per-level pending buffers fuses all levels in SBUF
(each node embedding is written to HBM exactly once, read never).
"""

import sys

for _p in ("/opt/trn_rl_repo",):
    if _p not in sys.path:
        sys.path.insert(0, _p)

import numpy as np

import concourse.bacc as bacc
import concourse.bass as bass
import concourse.mybir as mybir
from concourse import tile
from concourse.bass_utils import run_bass_kernel_spmd

DEPTH = 18
EMB = 128
HID = 256
VAL = 32
N_LEAVES = 2 ** DEPTH
N_NODES = 2 ** (DEPTH + 1) - 1
N_CORES = 8
SUB = DEPTH - 3              # per-core subtree: levels 0..SUB, 2^SUB leaves
ALPHA = 0.01                 # jax.nn.leaky_relu default negative_slope

F32 = mybir.dt.float32
F32R = mybir.dt.float32r
LRELU = mybir.ActivationFunctionType.Lrelu

# wpack column layout (all fp32r, [128, WPACK_COLS]):
_W1A = 0          # W1[0:128, :]    cols 0:256
_W1B = 256        # W1[128:256, :]  cols 256:512
_W2A = 512        # W2[0:128, :]    cols 512:640
_W2B = 640        # W2[128:256, :]  cols 640:768
_WE = 768         # We (rows 0:32)  cols 768:896
WPACK_COLS = 896
# bias tile columns ([128, 4] fp32): b1[0:128], b1[128:256], b2, be


def build_nc(sub=SUB, ch=512, wcap=1024, n_lv_dmas=4,
             act_func=None, strided_eo=True, f32r_dummies=True,
             use_dummies=False, f32r_min_n=2):
    """Build the per-core SPMD Bass program.

    sub:   subtree leaf level (leaves = 2^sub)
    ch:    leaf chunk width (columns per leaf matmul)
    wcap:  max pending-buffer width (columns consumed per upward step)
    """
    n_leaves = 2 ** sub
    n_out = 2 ** (sub + 1) - 1
    ch = min(ch, n_leaves)
    assert n_leaves % ch == 0
    n_chunks = n_leaves // ch
    assert n_chunks % n_lv_dmas == 0
    qs = n_leaves // n_lv_dmas          # leaf columns per input DMA
    chunks_per_q = n_chunks // n_lv_dmas

    def width(d):
        return min(wcap, 2 ** d)

    act = act_func if act_func is not None else LRELU

    nc = bacc.Bacc("TRN2", target_bir_lowering=False, debug=False)
    lv_d = nc.dram_tensor("lvT", [VAL, n_leaves], F32R, kind="ExternalInput").ap()
    wp_d = nc.dram_tensor("wpack", [128, WPACK_COLS], F32R, kind="ExternalInput").ap()
    bias_d = nc.dram_tensor("bias", [128, 4], F32, kind="ExternalInput").ap()
    out_d = nc.dram_tensor("outT", [EMB, n_out], F32R, kind="ExternalOutput").ap()

    with tile.TileContext(nc) as tc:
        import contextlib
        with contextlib.ExitStack() as ctx:
            const_pool = ctx.enter_context(tc.tile_pool(name="const", bufs=1))
            lv_pool = ctx.enter_context(tc.tile_pool(name="lv", bufs=2))
            pend_pool = ctx.enter_context(tc.tile_pool(name="pend", bufs=2))
            hs_pool = ctx.enter_context(tc.tile_pool(name="hs", bufs=2))
            ps_leaf = ctx.enter_context(tc.tile_pool(name="psl", bufs=2, space="PSUM"))
            ps_h = ctx.enter_context(tc.tile_pool(name="psh", bufs=2, space="PSUM"))
            ps_o = ctx.enter_context(tc.tile_pool(name="pso", bufs=1, space="PSUM"))
            ps_scr = ctx.enter_context(tc.tile_pool(name="pscr", bufs=1, space="PSUM"))

            wp = const_pool.tile([128, WPACK_COLS], F32R, tag="wp")
            nc.sync.dma_start(wp[:], wp_d[:])
            bias = const_pool.tile([128, 4], F32, tag="bias")
            nc.sync.dma_start(bias[:], bias_d[:])

            # Dummy matmuls absorb DMA-completion waits on the PE queue so
            # every real fp32r matmul needs at most one sync wait (walrus
            # limit for self-loading fp32 matmuls).  Each dummy writes a
            # distinct column of one persistent scratch PSUM tile, so the
            # dummies carry no dependency besides their DMA.
            if use_dummies:
                scr = ps_scr.tile([1, 1 + n_lv_dmas], F32, tag="scr")
            def dummy_mm(dst_col, a, b):
                if not use_dummies:
                    return
                if not f32r_dummies:
                    a = a.bitcast(F32); b = b.bitcast(F32)
                nc.tensor.matmul(scr[0:1, dst_col:dst_col + 1], a, b, start=True, stop=True)
            dummy_mm(0, wp[:, 0:1], wp[:, 1:2])

            # per-level pending buffers (binary-counter cascade)
            cur_tile = {d: None for d in range(sub + 1)}
            cur_fill = {d: 0 for d in range(sub + 1)}
            base_col = {d: 0 for d in range(sub + 1)}

            def emit(d, w):
                """Reserve w columns at level d; returns (tile, offset)."""
                wd = width(d)
                if cur_tile[d] is None:
                    cur_tile[d] = pend_pool.tile([128, wd], F32R, tag=f"p{d}", name=f"pend{d}")
                    cur_fill[d] = 0
                off = cur_fill[d]
                assert off + w <= wd
                cur_fill[d] = off + w
                return cur_tile[d], off

            def consume(d):
                """Pending buffer at level d is full: DMA it out and compute
                its parents into level d-1."""
                t = cur_tile[d]
                w = cur_fill[d]
                assert w == width(d)
                cur_tile[d] = None
                cur_fill[d] = 0
                b = base_col[d]
                base_col[d] = b + w
                off0 = 2 ** d - 1
                nc.sync.dma_start(out_d[:, off0 + b: off0 + b + w], t[:, 0:w])
                if d == 0:
                    return
                hw2 = w // 2
                if strided_eo:
                    E = t[:, 0:w:2]
                    O = t[:, 1:w:2]
                else:
                    E = t[:, 0:hw2]
                    O = t[:, hw2:w]
                # fp32r matmuls are ISA-illegal below a minimum moving size
                # (s3d3_mm_fp32r_restrictions); tiny top-of-tree steps use
                # plain fp32 instead.
                cast = (lambda ap: ap) if hw2 >= f32r_min_n else (lambda ap: ap.bitcast(F32))
                h_a = ps_h.tile([128, hw2], F32, tag="ha")
                h_b = ps_h.tile([128, hw2], F32, tag="hb")
                nc.tensor.matmul(h_a[:], cast(wp[:, 0:128]), cast(E), start=True, stop=False)
                nc.tensor.matmul(h_a[:], cast(wp[:, _W1B: _W1B + 128]), cast(O), start=False, stop=True)
                nc.tensor.matmul(h_b[:], cast(wp[:, 128:256]), cast(E), start=True, stop=False)
                nc.tensor.matmul(h_b[:], cast(wp[:, _W1B + 128: _W1B + 256]), cast(O), start=False, stop=True)
                ha_s = hs_pool.tile([128, hw2], F32R, tag="ha_s")
                hb_s = hs_pool.tile([128, hw2], F32R, tag="hb_s")
                nc.scalar.activation(ha_s[:], h_a[:], act, bias=bias[:, 0:1], alpha=ALPHA)
                nc.scalar.activation(hb_s[:], h_b[:], act, bias=bias[:, 1:2], alpha=ALPHA)
                o_p = ps_o.tile([128, hw2], F32, tag="op")
                nc.tensor.matmul(o_p[:], cast(wp[:, _W2A: _W2A + 128]), cast(ha_s[:]), start=True, stop=False)
                nc.tensor.matmul(o_p[:], cast(wp[:, _W2B: _W2B + 128]), cast(hb_s[:]), start=False, stop=True)
                dst, off = emit(d - 1, hw2)
                nc.scalar.activation(dst[:, off: off + hw2], o_p[:], act,
                                     bias=bias[:, 2:3], alpha=ALPHA)
                if cur_fill[d - 1] == width(d - 1):
                    consume(d - 1)

            qt = None
            for j in range(n_chunks):
                if j % chunks_per_q == 0:
                    qt = lv_pool.tile([VAL, qs], F32R, tag="qt")
                    q = j // chunks_per_q
                    nc.sync.dma_start(qt[:], lv_d[:, q * qs: (q + 1) * qs])
                    # absorb this quarter's DMA wait on the PE queue (see
                    # scratch dummy above)
                    dummy_mm(1 + q, qt[:, 0:1], qt[:, 1:2])
                m = j % chunks_per_q
                p = ps_leaf.tile([128, ch], F32, tag="pl")
                nc.tensor.matmul(p[:], wp[0:32, _WE: _WE + 128],
                                 qt[:, m * ch: (m + 1) * ch], start=True, stop=True)
                dst, off = emit(sub, ch)
                nc.scalar.activation(dst[:, off: off + ch], p[:], LRELU,
                                     bias=bias[:, 3:4], alpha=ALPHA)
                if cur_fill[sub] == width(sub):
                    consume(sub)

            assert all(cur_tile[d] is None for d in cur_tile), "unconsumed pending"
            assert all(base_col[d] == 2 ** d for d in base_col)

    # bacc passes: split multi-waits into event semaphores (HW allows one
    # sync wait per instruction), register allocation, DCE.
    nc.compile()
    return nc


def _leaky(v):
    return np.where(v >= 0, v, np.float32(ALPHA) * v).astype(np.float32)


def pack_weights(We, W1, W2):
    wpack = np.zeros((128, WPACK_COLS), np.float32)
    wpack[:, _W1A: _W1A + 256] = W1[0:128, :]
    wpack[:, _W1B: _W1B + 256] = W1[128:256, :]
    wpack[:, _W2A: _W2A + 128] = W2[0:128, :]
    wpack[:, _W2B: _W2B + 128] = W2[128:256, :]
    wpack[0:32, _WE: _WE + 128] = We
    return wpack


def pack_bias(b1, b2, be):
    bias = np.zeros((128, 4), np.float32)
    bias[:, 0] = b1[0:128]
    bias[:, 1] = b1[128:256]
    bias[:, 2] = b2
    bias[:, 3] = be
    return bias


_NC_CACHE = {}


def kernel(leaf_values, We, be, W1, b1, W2, b2, _trace=False):
    leaf_values = np.asarray(leaf_values, np.float32)
    We = np.asarray(We, np.float32)
    be = np.asarray(be, np.float32)
    W1 = np.asarray(W1, np.float32)
    b1 = np.asarray(b1, np.float32)
    W2 = np.asarray(W2, np.float32)
    b2 = np.asarray(b2, np.float32)

    sub_leaves = 2 ** SUB
    sub_nodes = 2 ** (SUB + 1) - 1

    wpack = pack_weights(We, W1, W2)
    bias = pack_bias(b1, b2, be)
    lvT = leaf_values.reshape(N_CORES, sub_leaves, VAL).transpose(0, 2, 1)
    in_maps = [
        {"lvT": np.ascontiguousarray(lvT[c]), "wpack": wpack, "bias": bias}
        for c in range(N_CORES)
    ]

    if "nc" not in _NC_CACHE:
        _NC_CACHE["nc"] = build_nc()
    nc = _NC_CACHE["nc"]

    res = run_bass_kernel_spmd(nc, in_maps, list(range(N_CORES)), trace=_trace)
    outs = [res.results[c]["outT"] for c in range(N_CORES)]

    embs = np.empty((N_NODES, EMB), np.float32)
    for c in range(N_CORES):
        full = np.ascontiguousarray(outs[c].T)        # [sub_nodes, 128]
        for d in range(SUB + 1):
            L = 3 + d
            n = 1 << d
            g0 = (1 << L) - 1 + c * n
            embs[g0: g0 + n] = full[n - 1: 2 * n - 1]

    # top 3 levels (nodes 0..6) on host
    lvl = np.stack([outs[c][:, 0] for c in range(N_CORES)])   # [8, 128]
    for l in (2, 1, 0):
        x = lvl.reshape(2 ** l, 2 * EMB)
        h = _leaky(x @ W1 + b1)
        lvl = _leaky(h @ W2 + b2)
        embs[(1 << l) - 1: (1 << (l + 1)) - 1] = lvl

    if _trace:
        kernel.last_results = res
    return embs


# revision 15
# speedup vs baseline: 1.2349x; 1.2349x over previous
"""Trainium2 Bass kernel for nn_Encoder_82910048682485 (binary-tree GNN encoder).

Structure exploited: in the heap-layout complete binary tree, the children of
the contiguous parent range [2^l-1, 2^(l+1)-1) are exactly the contiguous
range [2^(l+1)-1, 2^(l+2)-1), and parent p's children are cols 2s / 2s+1 of
that block.  So the whole computation is a chain of matmuls over shrinking
contiguous blocks — no real gather/scatter.

Sharding: data-parallel over the 8 subtrees rooted at nodes 7..14 (level 3).
Each core owns 2^15 leaves and computes its subtree's 2^16-1 node embeddings.
The top 7 nodes (levels 0..2) are computed on host (7 rows of a 256->128 MLP,
~0.001% of FLOPs).

On-chip layout is transposed: embeddings are stored [EMB=128 partitions,
nodes as free dim].  Then the even/odd child split needed by the cell MLP is
just a stride-2 free-dim access pattern, and each level-up step is 6 PE
matmuls + 2 leaky-relu passes.  Leaf chunks stream in and a binary-counter
cascade of per-level pending buffers fuses all levels in SBUF (each node
embedding is written to HBM exactly once, read back never).

Matmul operands are bf16 by default (fp32r runs in the PE's half-duty
fp32-HIGH mode and never warms the HAM clock gate; bf16 streams 1 row/cycle
at 2.4 GHz and halves the output DMA bytes).  PSUM accumulation stays fp32.
When all biases are zero (true for this model), leaky-relu work is split
between the Scalar engine (native Lrelu) and the idle Vector engine
(0.01*x + 0.99*relu(x), two fused ALU ops), and the two halves of the hidden
layer share one [128, w] PSUM tile so one pass covers both.
"""

import sys

for _p in ("/opt/trn_rl_repo",):
    if _p not in sys.path:
        sys.path.insert(0, _p)

import numpy as np

import concourse.bacc as bacc
import concourse.bass as bass
import concourse.mybir as mybir
from concourse import tile
from concourse.bass_utils import run_bass_kernel_spmd

DEPTH = 18
EMB = 128
HID = 256
VAL = 32
N_LEAVES = 2 ** DEPTH
N_NODES = 2 ** (DEPTH + 1) - 1
N_CORES = 8
SUB = DEPTH - 3              # per-core subtree: levels 0..SUB, 2^SUB leaves
ALPHA = 0.01                 # jax.nn.leaky_relu default negative_slope

F32 = mybir.dt.float32
F32R = mybir.dt.float32r
BF16 = mybir.dt.bfloat16
LRELU = mybir.ActivationFunctionType.Lrelu

# wpack column layout ([128, WPACK_COLS], matmul dtype):
_W1A = 0          # W1[0:128, :]    cols 0:256
_W1B = 256        # W1[128:256, :]  cols 256:512
_W2A = 512        # W2[0:128, :]    cols 512:640
_W2B = 640        # W2[128:256, :]  cols 640:768
_WE = 768         # We (rows 0:32)  cols 768:896
WPACK_COLS = 896
# bias tile columns ([128, 4] fp32): b1[0:128], b1[128:256], b2, be

# matmul operand precision (see module docstring)
MM_DT = BF16


def build_nc(sub=SUB, ch=1024, wcap=1024, n_lv_dmas=4,
             mm_dt=MM_DT, zero_bias=True, dve_out=True, dve_h_every=0,
             f32r_min_n=2):
    """Build the per-core SPMD Bass program.

    sub:        subtree leaf level (leaves = 2^sub)
    ch:         leaf chunk width (columns per leaf psum tile, <= 1024)
    wcap:       max pending-buffer width (columns consumed per upward step)
    zero_bias:  enables the fused single-pass h activation and DVE routing
                (only correct when b1 == 0)
    dve_out:    route the parent-output leaky-relu to the Vector engine
    dve_h_every: if k > 0, route every k-th h-activation to DVE as well
    """
    n_leaves = 2 ** sub
    n_out = 2 ** (sub + 1) - 1
    ch = min(ch, n_leaves)
    assert n_leaves % ch == 0
    n_chunks = n_leaves // ch
    assert n_chunks % n_lv_dmas == 0
    qs = n_leaves // n_lv_dmas          # leaf columns per input DMA
    chunks_per_q = n_chunks // n_lv_dmas

    def width(d):
        return min(wcap, 2 ** d)

    nc = bacc.Bacc("TRN2", target_bir_lowering=False, debug=False)
    lv_d = nc.dram_tensor("lvT", [VAL, n_leaves], mm_dt, kind="ExternalInput").ap()
    wp_d = nc.dram_tensor("wpack", [128, WPACK_COLS], mm_dt, kind="ExternalInput").ap()
    bias_d = nc.dram_tensor("bias", [128, 4], F32, kind="ExternalInput").ap()
    out_d = nc.dram_tensor("outT", [EMB, n_out], mm_dt, kind="ExternalOutput").ap()

    with tile.TileContext(nc) as tc:
        import contextlib
        with contextlib.ExitStack() as ctx:
            const_pool = ctx.enter_context(tc.tile_pool(name="const", bufs=1))
            lv_pool = ctx.enter_context(tc.tile_pool(name="lv", bufs=2))
            pend_pool = ctx.enter_context(tc.tile_pool(name="pend", bufs=2))
            hs_pool = ctx.enter_context(tc.tile_pool(name="hs", bufs=2))
            dvetmp_pool = ctx.enter_context(tc.tile_pool(name="dvetmp", bufs=2))
            # PSUM budget (8 banks): leaf [128,1024]x1 = 2, h [128,1024]x2 = 4,
            # o [128,512]x2 = 2.
            ps_leaf = ctx.enter_context(tc.tile_pool(name="psl", bufs=1, space="PSUM"))
            ps_h = ctx.enter_context(tc.tile_pool(name="psh", bufs=2, space="PSUM"))
            ps_o = ctx.enter_context(tc.tile_pool(name="pso", bufs=2, space="PSUM"))

            wp = const_pool.tile([128, WPACK_COLS], mm_dt, tag="wp")
            nc.sync.dma_start(wp[:], wp_d[:])
            bias = const_pool.tile([128, 4], F32, tag="bias")
            nc.sync.dma_start(bias[:], bias_d[:])

            def act_lrelu(dst_ap, src_ap, bias_col):
                nc.scalar.activation(dst_ap, src_ap, LRELU,
                                     bias=bias[:, bias_col: bias_col + 1],
                                     alpha=ALPHA)

            def dve_lrelu(dst_ap, src_ap, w):
                # dst = 0.01*x + 0.99*relu(x)  (zero-bias leaky-relu; PSUM may
                # be read only once per instruction, hence the two-op form)
                tmp = dvetmp_pool.tile([128, w], F32, tag="dvetmp", name="dvetmp")
                nc.vector.tensor_scalar(tmp[:], src_ap, 0.0, 1.0 - ALPHA,
                                        mybir.AluOpType.max, mybir.AluOpType.mult)
                nc.vector.scalar_tensor_tensor(dst_ap, src_ap, float(ALPHA), tmp[:],
                                               mybir.AluOpType.mult,
                                               mybir.AluOpType.add)

            # per-level pending buffers (binary-counter cascade)
            cur_tile = {d: None for d in range(sub + 1)}
            cur_fill = {d: 0 for d in range(sub + 1)}
            base_col = {d: 0 for d in range(sub + 1)}
            consume_ctr = {"n": 0}

            def emit(d, w):
                """Reserve w columns at level d; returns (tile, offset)."""
                wd = width(d)
                if cur_tile[d] is None:
                    cur_tile[d] = pend_pool.tile([128, wd], mm_dt,
                                                 tag=f"p{d}", name=f"pend{d}")
                    cur_fill[d] = 0
                off = cur_fill[d]
                assert off + w <= wd
                cur_fill[d] = off + w
                return cur_tile[d], off

            def consume(d):
                """Pending buffer at level d is full: DMA it out and compute
                its parents into level d-1."""
                t = cur_tile[d]
                w = cur_fill[d]
                assert w == width(d)
                cur_tile[d] = None
                cur_fill[d] = 0
                b = base_col[d]
                base_col[d] = b + w
                off0 = 2 ** d - 1
                nc.sync.dma_start(out_d[:, off0 + b: off0 + b + w], t[:, 0:w])
                if d == 0:
                    return
                consume_ctr["n"] += 1
                hw2 = w // 2
                E = t[:, 0:w:2]
                O = t[:, 1:w:2]
                # fp32r is ISA-illegal below a minimum moving size
                # (s3d3_mm_fp32r_restrictions); tiny steps fall back to fp32.
                cast = (lambda ap: ap) if (mm_dt != F32R or hw2 >= f32r_min_n) \
                    else (lambda ap: ap.bitcast(F32))
                h = ps_h.tile([128, w], F32, tag="h")
                nc.tensor.matmul(h[:, 0:hw2], cast(wp[:, 0:128]), cast(E),
                                 start=True, stop=False)
                nc.tensor.matmul(h[:, 0:hw2], cast(wp[:, _W1B: _W1B + 128]), cast(O),
                                 start=False, stop=True)
                nc.tensor.matmul(h[:, hw2:w], cast(wp[:, 128:256]), cast(E),
                                 start=True, stop=False)
                nc.tensor.matmul(h[:, hw2:w], cast(wp[:, _W1B + 128: _W1B + 256]),
                                 cast(O), start=False, stop=True)
                h_s = hs_pool.tile([128, w], mm_dt, tag="h_s")
                if zero_bias:
                    if dve_h_every and consume_ctr["n"] % dve_h_every == 0:
                        dve_lrelu(h_s[:], h[:], w)
                    else:
                        act_lrelu(h_s[:], h[:], 0)
                else:
                    act_lrelu(h_s[:, 0:hw2], h[:, 0:hw2], 0)
                    act_lrelu(h_s[:, hw2:w], h[:, hw2:w], 1)
                o_p = ps_o.tile([128, hw2], F32, tag="op")
                nc.tensor.matmul(o_p[:], cast(wp[:, _W2A: _W2A + 128]),
                                 cast(h_s[:, 0:hw2]), start=True, stop=False)
                nc.tensor.matmul(o_p[:], cast(wp[:, _W2B: _W2B + 128]),
                                 cast(h_s[:, hw2:w]), start=False, stop=True)
                dst, off = emit(d - 1, hw2)
                if zero_bias and dve_out:
                    dve_lrelu(dst[:, off: off + hw2], o_p[:], hw2)
                else:
                    act_lrelu(dst[:, off: off + hw2], o_p[:], 2)
                if cur_fill[d - 1] == width(d - 1):
                    consume(d - 1)

            qt = None
            for j in range(n_chunks):
                if j % chunks_per_q == 0:
                    qt = lv_pool.tile([VAL, qs], mm_dt, tag="qt")
                    q = j // chunks_per_q
                    nc.sync.dma_start(qt[:], lv_d[:, q * qs: (q + 1) * qs])
                m = j % chunks_per_q
                p = ps_leaf.tile([128, ch], F32, tag="pl")
                for s in range(0, ch, 512):
                    sw = min(512, ch - s)
                    nc.tensor.matmul(p[:, s: s + sw], wp[0:32, _WE: _WE + 128],
                                     qt[:, m * ch + s: m * ch + s + sw],
                                     start=True, stop=True)
                dst, off = emit(sub, ch)
                act_lrelu(dst[:, off: off + ch], p[:], 3)
                if cur_fill[sub] == width(sub):
                    consume(sub)

            assert all(cur_tile[d] is None for d in cur_tile), "unconsumed pending"
            assert all(base_col[d] == 2 ** d for d in base_col)

    # bacc passes: split multi-waits into event semaphores (HW allows one
    # sync wait per instruction), register allocation, DCE.
    nc.compile()
    return nc


def _leaky(v):
    return np.where(v >= 0, v, np.float32(ALPHA) * v).astype(np.float32)


def pack_weights(We, W1, W2):
    wpack = np.zeros((128, WPACK_COLS), np.float32)
    wpack[:, _W1A: _W1A + 256] = W1[0:128, :]
    wpack[:, _W1B: _W1B + 256] = W1[128:256, :]
    wpack[:, _W2A: _W2A + 128] = W2[0:128, :]
    wpack[:, _W2B: _W2B + 128] = W2[128:256, :]
    wpack[0:32, _WE: _WE + 128] = We
    return wpack


def pack_bias(b1, b2, be):
    bias = np.zeros((128, 4), np.float32)
    bias[:, 0] = b1[0:128]
    bias[:, 1] = b1[128:256]
    bias[:, 2] = b2
    bias[:, 3] = be
    return bias


def _np_dt(dt_):
    if dt_ == BF16:
        import ml_dtypes
        return ml_dtypes.bfloat16
    return np.float32


_NC_CACHE = {}


def kernel(leaf_values, We, be, W1, b1, W2, b2, _trace=False):
    leaf_values = np.asarray(leaf_values, np.float32)
    We = np.asarray(We, np.float32)
    be = np.asarray(be, np.float32)
    W1 = np.asarray(W1, np.float32)
    b1 = np.asarray(b1, np.float32)
    W2 = np.asarray(W2, np.float32)
    b2 = np.asarray(b2, np.float32)

    sub_leaves = 2 ** SUB

    npdt = _np_dt(MM_DT)
    zero_bias = not b1.any()
    wpack = pack_weights(We, W1, W2).astype(npdt)
    bias = pack_bias(b1, b2, be)
    lvT = leaf_values.reshape(N_CORES, sub_leaves, VAL).transpose(0, 2, 1)
    in_maps = [
        {"lvT": np.ascontiguousarray(lvT[c]).astype(npdt), "wpack": wpack,
         "bias": bias}
        for c in range(N_CORES)
    ]

    key = (MM_DT, zero_bias)
    if _NC_CACHE.get("key") != key:
        _NC_CACHE["nc"] = build_nc(mm_dt=MM_DT, zero_bias=zero_bias)
        _NC_CACHE["key"] = key
    nc = _NC_CACHE["nc"]

    res = run_bass_kernel_spmd(nc, in_maps, list(range(N_CORES)), trace=_trace)
    outs = [np.asarray(res.results[c]["outT"], np.float32) for c in range(N_CORES)]

    embs = np.empty((N_NODES, EMB), np.float32)
    for c in range(N_CORES):
        full = np.ascontiguousarray(outs[c].T)        # [sub_nodes, 128]
        for d in range(SUB + 1):
            L = 3 + d
            n = 1 << d
            g0 = (1 << L) - 1 + c * n
            embs[g0: g0 + n] = full[n - 1: 2 * n - 1]

    # top 3 levels (nodes 0..6) on host
    lvl = np.stack([outs[c][:, 0] for c in range(N_CORES)])   # [8, 128]
    for l in (2, 1, 0):
        x = lvl.reshape(2 ** l, 2 * EMB)
        h = _leaky(x @ W1 + b1)
        lvl = _leaky(h @ W2 + b2)
        embs[(1 << l) - 1: (1 << (l + 1)) - 1] = lvl

    if _trace:
        kernel.last_results = res
    return embs


# revision 20
# speedup vs baseline: 1.7875x; 1.4476x over previous
"""Trainium2 Bass kernel for nn_Encoder_82910048682485 (binary-tree GNN encoder).

Structure exploited: in the heap-layout complete binary tree, the children of
the contiguous parent range [2^l-1, 2^(l+1)-1) are exactly the contiguous
range [2^(l+1)-1, 2^(l+2)-1), and parent p's children are cols 2s / 2s+1 of
that block.  So the whole computation is a chain of matmuls over shrinking
contiguous blocks — no real gather/scatter.

Sharding: data-parallel over the 8 subtrees rooted at nodes 7..14 (level 3).
Each core owns 2^15 leaves and computes its subtree's 2^16-1 node embeddings.
The top 7 nodes (levels 0..2) are computed on host (7 rows of a 256->128 MLP,
~0.001% of FLOPs).

On-chip layout is transposed: embeddings are stored [EMB=128 partitions,
nodes as free dim].  Then the even/odd child split needed by the cell MLP is
just a stride-2 free-dim access pattern, and each level-up step is 6 PE
matmuls + 2 leaky-relu passes.  Leaf chunks stream in and a binary-counter
cascade of per-level pending buffers fuses all levels in SBUF (each node
embedding is written to HBM exactly once, read back never).

Matmul operands are bf16 by default (fp32r runs in the PE's half-duty
fp32-HIGH mode and never warms the HAM clock gate; bf16 streams 1 row/cycle
at 2.4 GHz and halves the output DMA bytes).  PSUM accumulation stays fp32.
When all biases are zero (true for this model), leaky-relu work is split
between the Scalar engine (native Lrelu) and the idle Vector engine
(0.01*x + 0.99*relu(x), two fused ALU ops), and the two halves of the hidden
layer share one [128, w] PSUM tile so one pass covers both.
"""

import sys

for _p in ("/opt/trn_rl_repo",):
    if _p not in sys.path:
        sys.path.insert(0, _p)

import numpy as np

import concourse.bacc as bacc
import concourse.bass as bass
import concourse.mybir as mybir
from concourse import tile
from concourse.bass_utils import run_bass_kernel_spmd

DEPTH = 18
EMB = 128
HID = 256
VAL = 32
N_LEAVES = 2 ** DEPTH
N_NODES = 2 ** (DEPTH + 1) - 1
N_CORES = 8
SUB = DEPTH - 3              # per-core subtree: levels 0..SUB, 2^SUB leaves
ALPHA = 0.01                 # jax.nn.leaky_relu default negative_slope

F32 = mybir.dt.float32
F32R = mybir.dt.float32r
BF16 = mybir.dt.bfloat16
LRELU = mybir.ActivationFunctionType.Lrelu

# wpack column layout ([128, WPACK_COLS], matmul dtype):
_W1A = 0          # W1[0:128, :]    cols 0:256
_W1B = 256        # W1[128:256, :]  cols 256:512
_W2A = 512        # W2[0:128, :]    cols 512:640
_W2B = 640        # W2[128:256, :]  cols 640:768
_WE = 768         # We (rows 0:32)  cols 768:896
WPACK_COLS = 896
# bias tile columns ([128, 4] fp32): b1[0:128], b1[128:256], b2, be

# matmul operand precision (see module docstring)
MM_DT = BF16


def build_nc(sub=SUB, ch=1024, wcap=1024, n_lv_dmas=4,
             mm_dt=MM_DT, zero_bias=True, dve_out=True, dve_h_every=0,
             f32r_min_n=2):
    """Build the per-core SPMD Bass program.

    sub:        subtree leaf level (leaves = 2^sub)
    ch:         leaf chunk width (columns per leaf psum tile, <= 1024)
    wcap:       max pending-buffer width (columns consumed per upward step)
    zero_bias:  enables the fused single-pass h activation and DVE routing
                (only correct when b1 == 0)
    dve_out:    route the parent-output leaky-relu to the Vector engine
    dve_h_every: if k > 0, route every k-th h-activation to DVE as well
    """
    n_leaves = 2 ** sub
    n_out = 2 ** (sub + 1) - 1
    ch = min(ch, n_leaves)
    assert n_leaves % ch == 0
    n_chunks = n_leaves // ch
    assert n_chunks % n_lv_dmas == 0
    qs = n_leaves // n_lv_dmas          # leaf columns per input DMA
    chunks_per_q = n_chunks // n_lv_dmas

    def width(d):
        return min(wcap, 2 ** d)

    nc = bacc.Bacc("TRN2", target_bir_lowering=False, debug=False)
    lv_d = nc.dram_tensor("lvT", [VAL, n_leaves], mm_dt, kind="ExternalInput").ap()
    wp_d = nc.dram_tensor("wpack", [128, WPACK_COLS], mm_dt, kind="ExternalInput").ap()
    bias_d = nc.dram_tensor("bias", [128, 4], F32, kind="ExternalInput").ap()
    out_d = nc.dram_tensor("outT", [EMB, n_out], mm_dt, kind="ExternalOutput").ap()

    with tile.TileContext(nc) as tc:
        import contextlib
        with contextlib.ExitStack() as ctx:
            const_pool = ctx.enter_context(tc.tile_pool(name="const", bufs=1))
            lv_pool = ctx.enter_context(tc.tile_pool(name="lv", bufs=2))
            pend_pool = ctx.enter_context(tc.tile_pool(name="pend", bufs=3))
            hs_pool = ctx.enter_context(tc.tile_pool(name="hs", bufs=3))
            dvetmp_pool = ctx.enter_context(tc.tile_pool(name="dvetmp", bufs=3))
            # PSUM budget (8 banks): leaf [128,1024]x1 = 2, h [128,1024]x2 = 4,
            # o [128,512]x2 = 2.
            ps_leaf = ctx.enter_context(tc.tile_pool(name="psl", bufs=1, space="PSUM"))
            ps_h = ctx.enter_context(tc.tile_pool(name="psh", bufs=2, space="PSUM"))
            ps_o = ctx.enter_context(tc.tile_pool(name="pso", bufs=2, space="PSUM"))

            wp = const_pool.tile([128, WPACK_COLS], mm_dt, tag="wp")
            nc.sync.dma_start(wp[:], wp_d[:])
            bias = const_pool.tile([128, 4], F32, tag="bias")
            nc.sync.dma_start(bias[:], bias_d[:])

            def act_lrelu(dst_ap, src_ap, bias_col):
                nc.scalar.activation(dst_ap, src_ap, LRELU,
                                     bias=bias[:, bias_col: bias_col + 1],
                                     alpha=ALPHA)

            def dve_lrelu(dst_ap, src_ap, w):
                # dst = 0.01*x + 0.99*relu(x)  (zero-bias leaky-relu; PSUM may
                # be read only once per instruction, hence the two-op form)
                tmp = dvetmp_pool.tile([128, w], F32, tag="dvetmp", name="dvetmp")
                nc.vector.tensor_scalar(tmp[:], src_ap, 0.0, 1.0 - ALPHA,
                                        mybir.AluOpType.max, mybir.AluOpType.mult)
                nc.vector.scalar_tensor_tensor(dst_ap, src_ap, float(ALPHA), tmp[:],
                                               mybir.AluOpType.mult,
                                               mybir.AluOpType.add)

            # per-level pending buffers (binary-counter cascade)
            cur_tile = {d: None for d in range(sub + 1)}
            cur_fill = {d: 0 for d in range(sub + 1)}
            base_col = {d: 0 for d in range(sub + 1)}
            consume_ctr = {"n": 0}

            def emit(d, w):
                """Reserve w columns at level d; returns (tile, offset)."""
                wd = width(d)
                if cur_tile[d] is None:
                    cur_tile[d] = pend_pool.tile([128, wd], mm_dt,
                                                 tag=f"p{d}", name=f"pend{d}")
                    cur_fill[d] = 0
                off = cur_fill[d]
                assert off + w <= wd
                cur_fill[d] = off + w
                return cur_tile[d], off

            def queue_full(d):
                """Detach level d's (full) pending tile and queue its consume."""
                assert cur_fill[d] == width(d)
                ready.append((d, cur_tile[d], cur_fill[d]))
                cur_tile[d] = None
                cur_fill[d] = 0

            def consume(d, t, w):
                """DMA a full level-d tile out and compute its parents into
                level d-1."""
                b = base_col[d]
                base_col[d] = b + w
                off0 = 2 ** d - 1
                nc.sync.dma_start(out_d[:, off0 + b: off0 + b + w], t[:, 0:w])
                if d == 0:
                    return
                consume_ctr["n"] += 1
                hw2 = w // 2
                E = t[:, 0:w:2]
                O = t[:, 1:w:2]
                # fp32r is ISA-illegal below a minimum moving size
                # (s3d3_mm_fp32r_restrictions); tiny steps fall back to fp32.
                cast = (lambda ap: ap) if (mm_dt != F32R or hw2 >= f32r_min_n) \
                    else (lambda ap: ap.bitcast(F32))
                h = ps_h.tile([128, w], F32, tag="h")
                nc.tensor.matmul(h[:, 0:hw2], cast(wp[:, 0:128]), cast(E),
                                 start=True, stop=False)
                nc.tensor.matmul(h[:, 0:hw2], cast(wp[:, _W1B: _W1B + 128]), cast(O),
                                 start=False, stop=True)
                nc.tensor.matmul(h[:, hw2:w], cast(wp[:, 128:256]), cast(E),
                                 start=True, stop=False)
                nc.tensor.matmul(h[:, hw2:w], cast(wp[:, _W1B + 128: _W1B + 256]),
                                 cast(O), start=False, stop=True)
                h_s = hs_pool.tile([128, w], mm_dt, tag="h_s")
                if zero_bias:
                    if dve_h_every and consume_ctr["n"] % dve_h_every == 0:
                        dve_lrelu(h_s[:], h[:], w)
                    else:
                        act_lrelu(h_s[:], h[:], 0)
                else:
                    act_lrelu(h_s[:, 0:hw2], h[:, 0:hw2], 0)
                    act_lrelu(h_s[:, hw2:w], h[:, hw2:w], 1)
                o_p = ps_o.tile([128, hw2], F32, tag="op")
                nc.tensor.matmul(o_p[:], cast(wp[:, _W2A: _W2A + 128]),
                                 cast(h_s[:, 0:hw2]), start=True, stop=False)
                nc.tensor.matmul(o_p[:], cast(wp[:, _W2B: _W2B + 128]),
                                 cast(h_s[:, hw2:w]), start=False, stop=True)
                dst, off = emit(d - 1, hw2)
                if zero_bias and dve_out:
                    dve_lrelu(dst[:, off: off + hw2], o_p[:], hw2)
                else:
                    act_lrelu(dst[:, off: off + hw2], o_p[:], 2)
                if cur_fill[d - 1] == width(d - 1):
                    queue_full(d - 1)

            # Deferred-consume queue: running a full cascade inline would put
            # a chain of dependent instructions at the head of the in-order
            # PE queue and stall it.  Instead, when a pending buffer fills it
            # is detached and queued, and one consume is drained per leaf
            # chunk — by then its inputs are a full chunk old, so the PE
            # never waits.
            ready = []

            def drain(n):
                for _ in range(n):
                    if not ready:
                        return
                    dd, t, f = ready.pop(0)
                    consume(dd, t, f)

            qt = None
            for j in range(n_chunks):
                if j % chunks_per_q == 0:
                    qt = lv_pool.tile([VAL, qs], mm_dt, tag="qt")
                    q = j // chunks_per_q
                    nc.sync.dma_start(qt[:], lv_d[:, q * qs: (q + 1) * qs])
                m = j % chunks_per_q
                p = ps_leaf.tile([128, ch], F32, tag="pl")
                for s in range(0, ch, 512):
                    sw = min(512, ch - s)
                    nc.tensor.matmul(p[:, s: s + sw], wp[0:32, _WE: _WE + 128],
                                     qt[:, m * ch + s: m * ch + s + sw],
                                     start=True, stop=True)
                dst, off = emit(sub, ch)
                act_lrelu(dst[:, off: off + ch], p[:], 3)
                if cur_fill[sub] == width(sub):
                    queue_full(sub)
                drain(1)
                # after cascade bursts, keep the backlog short so pending-
                # buffer slots recycle before the next fill needs them
                if len(ready) > 3:
                    drain(len(ready) - 3)
            while ready:
                drain(1)

            assert all(cur_tile[d] is None for d in cur_tile), "unconsumed pending"
            assert all(base_col[d] == 2 ** d for d in base_col)

    # bacc passes: split multi-waits into event semaphores (HW allows one
    # sync wait per instruction), register allocation, DCE.
    nc.compile()
    return nc


def _leaky(v):
    return np.where(v >= 0, v, np.float32(ALPHA) * v).astype(np.float32)


def pack_weights(We, W1, W2):
    wpack = np.zeros((128, WPACK_COLS), np.float32)
    wpack[:, _W1A: _W1A + 256] = W1[0:128, :]
    wpack[:, _W1B: _W1B + 256] = W1[128:256, :]
    wpack[:, _W2A: _W2A + 128] = W2[0:128, :]
    wpack[:, _W2B: _W2B + 128] = W2[128:256, :]
    wpack[0:32, _WE: _WE + 128] = We
    return wpack


def pack_bias(b1, b2, be):
    bias = np.zeros((128, 4), np.float32)
    bias[:, 0] = b1[0:128]
    bias[:, 1] = b1[128:256]
    bias[:, 2] = b2
    bias[:, 3] = be
    return bias


def _np_dt(dt_):
    if dt_ == BF16:
        import ml_dtypes
        return ml_dtypes.bfloat16
    return np.float32


_NC_CACHE = {}


def kernel(leaf_values, We, be, W1, b1, W2, b2, _trace=False):
    leaf_values = np.asarray(leaf_values, np.float32)
    We = np.asarray(We, np.float32)
    be = np.asarray(be, np.float32)
    W1 = np.asarray(W1, np.float32)
    b1 = np.asarray(b1, np.float32)
    W2 = np.asarray(W2, np.float32)
    b2 = np.asarray(b2, np.float32)

    sub_leaves = 2 ** SUB

    npdt = _np_dt(MM_DT)
    zero_bias = not b1.any()
    wpack = pack_weights(We, W1, W2).astype(npdt)
    bias = pack_bias(b1, b2, be)
    lvT = leaf_values.reshape(N_CORES, sub_leaves, VAL).transpose(0, 2, 1)
    in_maps = [
        {"lvT": np.ascontiguousarray(lvT[c]).astype(npdt), "wpack": wpack,
         "bias": bias}
        for c in range(N_CORES)
    ]

    key = (MM_DT, zero_bias)
    if _NC_CACHE.get("key") != key:
        _NC_CACHE["nc"] = build_nc(mm_dt=MM_DT, zero_bias=zero_bias)
        _NC_CACHE["key"] = key
    nc = _NC_CACHE["nc"]

    res = run_bass_kernel_spmd(nc, in_maps, list(range(N_CORES)), trace=_trace)
    outs = [np.asarray(res.results[c]["outT"], np.float32) for c in range(N_CORES)]

    embs = np.empty((N_NODES, EMB), np.float32)
    for c in range(N_CORES):
        full = np.ascontiguousarray(outs[c].T)        # [sub_nodes, 128]
        for d in range(SUB + 1):
            L = 3 + d
            n = 1 << d
            g0 = (1 << L) - 1 + c * n
            embs[g0: g0 + n] = full[n - 1: 2 * n - 1]

    # top 3 levels (nodes 0..6) on host
    lvl = np.stack([outs[c][:, 0] for c in range(N_CORES)])   # [8, 128]
    for l in (2, 1, 0):
        x = lvl.reshape(2 ** l, 2 * EMB)
        h = _leaky(x @ W1 + b1)
        lvl = _leaky(h @ W2 + b2)
        embs[(1 << l) - 1: (1 << (l + 1)) - 1] = lvl

    if _trace:
        kernel.last_results = res
    return embs
